# revision 9
# baseline (speedup 1.0000x reference)
"""Trainium2 Bass kernel for nn_CategoricalGumbelSoftmax.

Math (per reference):
  hs = gene_emb @ W1s^T ; ht = gene_emb @ W1t^T
  h[b,s,t,:] = relu(hs[b,s,:] + ht[b,t,:] + b1)
  logits = h @ W2^T + b2 ; z = logits + (-log(-log u))
  y = one_hot(argmax z) (forward value of the straight-through estimator)
  M = y[...,2] - y[...,0]

Sharding: src-gene axis s split across 8 cores (64 src genes x 2 batches
per core = 128 (b,s) rows). Weights replicated.

Per-core device pipeline:
  A: project ht^T (d-on-partition, t-free) and hs^T via PE; fold b1 into ht^T.
  B: h = relu(htT + hs_col) per (b,s) -- per-partition-scalar add+max on
     DVE (tensor_scalar dual-op) / ACT (Relu with bias) / GPSIMD, split.
  C: z = W2p^T @ h on PE, fp32r moving, 4 (b,s) instances col-packed per
     PSUM bank via tile_position.
  D: PSUM -> SBUF copy -> DMA to DRAM z planes [c, (b,s), t].
  E: gumbel from u planes (2x Ln on ACT), z~ = Z + b2 - log(-log u),
     argmax via max+is_equal, M = y2 - y0. DMA out.
"""
import numpy as np

import concourse.bacc as bacc
import concourse.mybir as mybir
from concourse.tile import TileContext
from concourse.bass_utils import run_bass_kernel_spmd

dt = mybir.dt
F32 = dt.float32
F32R = dt.float32r
AF = mybir.ActivationFunctionType
OP = mybir.AluOpType

B, G, D, C = 2, 512, 256, 3
N_CORES = 8
S = G // N_CORES          # 64 src genes per core per batch
P = B * S                 # 128 = (b,s) rows per core
T = G                     # 512 targets
KC = D // 128             # 2 contraction chunks

# engine rotation for the relu stage: per (b,s) instance, chunk0 engine,
# chunk1 engine.  v=DVE, a=ACT, g=GPSIMD.
RELU_PATTERN = ["va", "vg", "av", "gv"]


def build_program(b2_vals, sim_safe=False):
    nc = bacc.Bacc(None, target_bir_lowering=False)

    geT = nc.dram_tensor("geT", [B, D, T], F32, kind="ExternalInput")
    geS = nc.dram_tensor("geS", [B, D, S], F32, kind="ExternalInput")
    w1sT = nc.dram_tensor("w1sT", [D, D], F32, kind="ExternalInput")
    w1tT = nc.dram_tensor("w1tT", [D, D], F32, kind="ExternalInput")
    w2tp = nc.dram_tensor("w2tp", [D, 32], F32, kind="ExternalInput")
    b1c = nc.dram_tensor("b1c", [D, 1], F32, kind="ExternalInput")
    u_pl = nc.dram_tensor("u_pl", [C, P, T], F32, kind="ExternalInput")
    y_pl = nc.dram_tensor("y_pl", [C, P, T], F32, kind="ExternalOutput")
    m_pl = nc.dram_tensor("m_pl", [P, T], F32, kind="ExternalOutput")

    with TileContext(nc) as tc:
        with (
            tc.tile_pool(name="wsb", bufs=1) as wsb,        # weights/const/inputs
            tc.tile_pool(name="hts", bufs=1) as hts,        # htT/hsT SBUF
            tc.tile_pool(name="hpool", bufs=10) as hpool,   # h working tiles
            tc.tile_pool(name="zdrain", bufs=3) as zdrain,  # PSUM drain tiles
            tc.tile_pool(name="epool", bufs=1) as epool,    # stage E tiles
            tc.tile_pool(name="psA", bufs=2, space="PSUM") as psA,
            tc.tile_pool(name="psAs", bufs=2, space="PSUM") as psAs,
            tc.tile_pool(name="psZ", bufs=4, space="PSUM") as psZ,
            tc.tile_pool(name="zdram", bufs=1, space="DRAM") as zdram,
        ):
            # ---- input loads ----
            ge_sb = [[wsb.tile([128, T], F32, name=f"ge{b}{k}", tag=f"ge{b}{k}") for k in range(KC)]
                     for b in range(B)]
            geS_sb = [[wsb.tile([128, S], F32, name=f"gs{b}{k}", tag=f"gs{b}{k}") for k in range(KC)]
                      for b in range(B)]
            w1s_sb = [[wsb.tile([128, 128], F32, name=f"w1s{k}{m}", tag=f"w1s{k}{m}") for m in range(KC)]
                      for k in range(KC)]
            w1t_sb = [[wsb.tile([128, 128], F32, name=f"w1t{k}{m}", tag=f"w1t{k}{m}") for m in range(KC)]
                      for k in range(KC)]
            w2_sb = [wsb.tile([128, 32], F32, name=f"w2{k}", tag=f"w2{k}") for k in range(KC)]
            b1_sb = [wsb.tile([128, 1], F32, name=f"b1{k}", tag=f"b1{k}") for k in range(KC)]
            for b in range(B):
                for k in range(KC):
                    nc.sync.dma_start(out=ge_sb[b][k][:].bitcast(F32R),
                                      in_=geT[b, 128 * k:128 * (k + 1), :].bitcast(F32R))
                    nc.sync.dma_start(out=geS_sb[b][k][:].bitcast(F32R),
                                      in_=geS[b, 128 * k:128 * (k + 1), :].bitcast(F32R))
            for k in range(KC):
                for m in range(KC):
                    nc.sync.dma_start(
                        out=w1s_sb[k][m][:].bitcast(F32R),
                        in_=w1sT[128 * k:128 * (k + 1), 128 * m:128 * (m + 1)].bitcast(F32R))
                    nc.sync.dma_start(
                        out=w1t_sb[k][m][:].bitcast(F32R),
                        in_=w1tT[128 * k:128 * (k + 1), 128 * m:128 * (m + 1)].bitcast(F32R))
                nc.sync.dma_start(out=w2_sb[k][:].bitcast(F32R),
                                  in_=w2tp[128 * k:128 * (k + 1), :].bitcast(F32R))
                nc.sync.dma_start(out=b1_sb[k][:],
                                  in_=b1c[128 * k:128 * (k + 1), :])

            # stage E inputs have no deps -- load them up front too
            u_sb = [epool.tile([P, T], F32, name=f"u{c}", tag=f"u{c}") for c in range(C)]
            for c in range(C):
                nc.sync.dma_start(out=u_sb[c][:], in_=u_pl[c])

            # ---- stage A: projections ----
            # htT_sb[b][m] = (W1t^T chunk).T @ geT[b]  + b1   (128 dout x 512 t)
            htT_sb = [[hts.tile([128, T], F32, name=f"ht{b}{m}", tag=f"ht{b}{m}") for m in range(KC)]
                      for b in range(B)]
            # hsT_sb[b][m][:, s] = per-partition bias column for (b, s)
            hsT_sb = [[hts.tile([128, S], F32, name=f"hs{b}{m}", tag=f"hs{b}{m}") for m in range(KC)]
                      for b in range(B)]
            for b in range(B):
                for m in range(KC):
                    pht = psA.tile([128, T], F32, name="pht", tag="pht")
                    for k in range(KC):
                        nc.tensor.matmul(pht[:], w1t_sb[k][m][:].bitcast(F32R),
                                         ge_sb[b][k][:].bitcast(F32R),
                                         start=(k == 0), stop=(k == KC - 1))
                    # fold b1 while copying out of PSUM
                    nc.scalar.activation(htT_sb[b][m][:], pht[:], AF.Identity,
                                         bias=b1_sb[m][:], scale=1.0)
                    phs = psAs.tile([128, S], F32, name="phs", tag="phs")
                    for k in range(KC):
                        nc.tensor.matmul(phs[:], w1s_sb[k][m][:].bitcast(F32R),
                                         geS_sb[b][k][:].bitcast(F32R),
                                         start=(k == 0), stop=(k == KC - 1))
                    nc.vector.tensor_copy(hsT_sb[b][m][:], phs[:])

            # ---- stages B + C + D: relu, z-matmul, drain ----
            # fp32r matmuls require dst base partition 0: one (b,s) instance
            # per PSUM tile, z rows 0..2.
            z_t = zdram.tile([C, P, T], F32, name="zt", tag="zt")
            for bs in range(P):
                b, s = bs // S, bs % S
                pat = RELU_PATTERN[bs % len(RELU_PATTERN)]
                htiles = []
                for m in range(KC):
                    h = hpool.tile([128, T], F32, name="h", tag="h")
                    col = hsT_sb[b][m][:, s:s + 1]
                    eng = pat[m]
                    if eng == "v":
                        nc.vector.tensor_scalar(h[:].bitcast(F32R),
                                                htT_sb[b][m][:], col,
                                                0.0, OP.add, OP.max)
                    elif eng == "g":
                        nc.gpsimd.tensor_scalar(h[:].bitcast(F32R),
                                                htT_sb[b][m][:], col,
                                                0.0, OP.add, OP.max)
                    else:
                        nc.scalar.activation(h[:].bitcast(F32R),
                                             htT_sb[b][m][:], AF.Relu,
                                             bias=col, scale=1.0)
                    htiles.append(h)
                pz = psZ.tile([32, T], F32, name="pz", tag="pz")
                for m in range(KC):
                    nc.tensor.matmul(pz[0:32, :],
                                     w2_sb[m][:].bitcast(F32R),
                                     htiles[m][:].bitcast(F32R),
                                     start=(m == 0), stop=(m == KC - 1))
                zsb = zdrain.tile([C, T], F32, name="zsb", tag="zsb")
                if bs % 2 == 0:
                    nc.vector.tensor_copy(zsb[:], pz[0:C, :])
                else:
                    nc.scalar.copy(zsb[:], pz[0:C, :])
                nc.sync.dma_start(out=z_t[:, bs:bs + 1, :], in_=zsb[:])

            # ---- stage E ----
            q_sb = []
            for c in range(C):
                tln = epool.tile([P, T], F32, name=f"tln{c}", tag=f"tln{c}")
                nc.scalar.activation(tln[:], u_sb[c][:], AF.Ln)
                q = epool.tile([P, T], F32, name=f"q{c}", tag=f"q{c}")
                nc.scalar.activation(q[:], tln[:], AF.Ln, scale=-1.0)
                q_sb.append(q)
            zt_sb = []
            for c in range(C):
                zc = epool.tile([P, T], F32, name=f"z{c}", tag=f"z{c}")
                nc.sync.dma_start(out=zc[:], in_=z_t[c])
                ztc = epool.tile([P, T], F32, name=f"zt{c}", tag=f"zt{c}")
                # (Z + b2_c) - log(-log u)
                nc.vector.scalar_tensor_tensor(ztc[:], zc[:], float(b2_vals[c]),
                                               q_sb[c][:], OP.add, OP.subtract)
                zt_sb.append(ztc)
            zm = epool.tile([P, T], F32, name="zm", tag="zm")
            nc.vector.tensor_tensor(zm[:], zt_sb[0][:], zt_sb[1][:], OP.max)
            nc.vector.tensor_tensor(zm[:], zm[:], zt_sb[2][:], OP.max)
            y_sb = []
            for c in range(C):
                yc = epool.tile([P, T], F32, name=f"y{c}", tag=f"y{c}")
                nc.vector.tensor_tensor(yc[:], zt_sb[c][:], zm[:], OP.is_equal)
                nc.sync.dma_start(out=y_pl[c], in_=yc[:])
                y_sb.append(yc)
            mm = epool.tile([P, T], F32, name="mm", tag="mm")
            nc.vector.tensor_tensor(mm[:], y_sb[2][:], y_sb[0][:], OP.subtract)
            nc.sync.dma_start(out=m_pl[:], in_=mm[:])

    nc.finalize()
    return nc


def _prep_inputs(gene_emb, u, W1, b1, W2):
    geT = np.ascontiguousarray(gene_emb.transpose(0, 2, 1))        # (B, D, G)
    w1sT = np.ascontiguousarray(W1[:, :D].T)                       # (din, dout)
    w1tT = np.ascontiguousarray(W1[:, D:].T)
    w2tp = np.zeros((D, 32), np.float32)
    w2tp[:, :C] = W2.T
    b1col = np.ascontiguousarray(b1.reshape(D, 1))
    uP = np.ascontiguousarray(u.transpose(3, 0, 1, 2))             # (C, B, G, G)
    in_maps = []
    for k in range(N_CORES):
        sl = slice(k * S, (k + 1) * S)
        in_maps.append({
            "geT": geT,
            "geS": np.ascontiguousarray(geT[:, :, sl]),
            "w1sT": w1sT,
            "w1tT": w1tT,
            "w2tp": w2tp,
            "b1c": b1col,
            "u_pl": np.ascontiguousarray(uP[:, :, sl, :]).reshape(C, P, T),
        })
    return in_maps


def _gather(results):
    M = np.empty((B, G, G), np.float32)
    y = np.empty((B, G, G, C), np.float32)
    for k in range(N_CORES):
        ypl = results[k]["y_pl"]          # (C, P, T)
        mpl = results[k]["m_pl"]          # (P, T)
        for b in range(B):
            M[b, k * S:(k + 1) * S] = mpl[b * S:(b + 1) * S]
            y[b, k * S:(k + 1) * S] = ypl[:, b * S:(b + 1) * S, :].transpose(1, 2, 0)
    return M, y


_prog_cache = {}


def kernel(gene_emb, u, W1, b1, W2, b2, _trace=False, _trace_kwargs=None):
    key = tuple(np.asarray(b2, np.float32).tolist())
    if key not in _prog_cache:
        _prog_cache[key] = build_program(np.asarray(b2, np.float32))
    nc = _prog_cache[key]
    in_maps = _prep_inputs(np.asarray(gene_emb, np.float32),
                           np.asarray(u, np.float32),
                           np.asarray(W1, np.float32),
                           np.asarray(b1, np.float32),
                           np.asarray(W2, np.float32))
    kw = {}
    if _trace:
        kw = dict(trace=True, **(_trace_kwargs or {}))
    res = run_bass_kernel_spmd(nc, in_maps, list(range(N_CORES)), **kw)
    M, y = _gather(res.results)
    if _trace:
        return (M, y), res
    return M, y


# revision 14
# speedup vs baseline: 1.7450x; 1.7450x over previous
"""Trainium2 Bass kernel for nn_CategoricalGumbelSoftmax.

Math (per reference):
  hs = gene_emb @ W1s^T ; ht = gene_emb @ W1t^T (+b1 folded into ht)
  h[b,s,t,:] = relu(hs[b,s,:] + ht[b,t,:])
  z = h @ W2^T + b2 + (-log(-log u));  y = one_hot(argmax z);  M = y2 - y0

Sharding: src-gene axis split across 8 cores (64 src x 2 batches = 128
(b,s) instances per core). Weights replicated.

Device pipeline (per core):
  A: PE projects htT (d-part, t-free; b1 folded) and hsT; htT also goes
     to DRAM for replication; hs (negated + plain) to DRAM for gathers.
  R: htT replicated x8 along partitions -> htRep tiles (128 = 16d x 8
     instances, 512) via DMA.
  B: relu for 8-instance groups, partition = (i inst, l d-of-16).
     ACT groups: h = Relu(htRep + hsCol)  (exact relu w/ bias)
     DVE/GPS groups: h' = max(htRep, -hsCol) = h - hs (fast TT with a
     free-broadcast column; fp32r write). The missing +hs term is
     rank-1 in d and is added in stage E via hsw = W2 @ hs (exact).
  C: z' = blockdiag(W2)^T @ h on PE (fp32r, K = 8 inst x 16 d, M = 24),
     16 chunk-matmuls accumulate per group -> PSUM (24, 512).
  D: PSUM -> SBUF copy -> DMA to DRAM z planes [c, (b,s), t].
  E: gumbel (2x Ln on ACT), z~ = Z + mask*hsw + b2 - log(-log u),
     argmax via max + is_equal, M = y2 - y0.
"""
import numpy as np

import concourse.bacc as bacc
import concourse.mybir as mybir
from concourse.tile import TileContext
from concourse.bass_utils import run_bass_kernel_spmd

dt = mybir.dt
F32 = dt.float32
F32R = dt.float32r
AF = mybir.ActivationFunctionType
OP = mybir.AluOpType

B, G, D, C = 2, 512, 256, 3
N_CORES = 8
S = G // N_CORES          # 64 src genes per core per batch
P = B * S                 # 128 (b,s) instances per core
T = G                     # 512 targets
KC = D // 128             # 2 x 128 contraction chunks (stage A)
R = 8                     # instances packed per matmul column (stage C)
L = 128 // R              # 16 d-values per instance per column
NCH = D // L              # 16 K-chunks per group
NG = P // R               # 16 groups
MW = C * R                # 24 used output rows per group

# Per-group engine for the relu stage: 'a' = ACT (exact relu),
# 'v' = DVE / 'g' = GPSIMD (max-form, needs hsw correction in stage E).
GROUP_ENG = list("avavavavavavavav")


def build_program(b2_vals, sim_safe=False):
    nc = bacc.Bacc(None, target_bir_lowering=False)

    geT = nc.dram_tensor("geT", [B, D, T], F32, kind="ExternalInput")
    geS = nc.dram_tensor("geS", [B, D, S], F32, kind="ExternalInput")
    w1sT = nc.dram_tensor("w1sT", [D, D], F32, kind="ExternalInput")
    w1tT = nc.dram_tensor("w1tT", [D, D], F32, kind="ExternalInput")
    w2tp = nc.dram_tensor("w2tp", [D, 32], F32, kind="ExternalInput")
    w2blk = nc.dram_tensor("w2blk", [NCH, 128, 32], F32, kind="ExternalInput")
    b1c = nc.dram_tensor("b1c", [D, 1], F32, kind="ExternalInput")
    hswmask = nc.dram_tensor("hswmask", [P, 1], F32, kind="ExternalInput")
    u_pl = nc.dram_tensor("u_pl", [C, P, T], F32, kind="ExternalInput")
    y_pl = nc.dram_tensor("y_pl", [C, P, T], F32, kind="ExternalOutput")
    m_pl = nc.dram_tensor("m_pl", [P, T], F32, kind="ExternalOutput")

    with TileContext(nc) as tc:
        with (
            tc.tile_pool(name="wsb", bufs=1) as wsb,
            tc.tile_pool(name="hts", bufs=1) as hts,
            tc.tile_pool(name="hrep", bufs=4) as hrep,
            tc.tile_pool(name="hpool", bufs=8) as hpool,
            tc.tile_pool(name="hcolp", bufs=9) as hcolp,
            tc.tile_pool(name="zdrain", bufs=3) as zdrain,
            tc.tile_pool(name="epool", bufs=1) as epool,
            tc.tile_pool(name="ps", bufs=1, space="PSUM") as psp,
            tc.tile_pool(name="dscratch", bufs=1, space="DRAM") as dscratch,
        ):
            # ---- input loads ----
            ge_sb = [[wsb.tile([128, T], F32, name=f"ge{b}{k}") for k in range(KC)]
                     for b in range(B)]
            geS_sb = [[wsb.tile([128, S], F32, name=f"gs{b}{k}") for k in range(KC)]
                      for b in range(B)]
            w1s_sb = [[wsb.tile([128, 128], F32, name=f"w1s{k}{m}") for m in range(KC)]
                      for k in range(KC)]
            w1t_sb = [[wsb.tile([128, 128], F32, name=f"w1t{k}{m}") for m in range(KC)]
                      for k in range(KC)]
            w2_sb = [wsb.tile([128, 32], F32, name=f"w2f{k}") for k in range(KC)]
            w2b_sb = [wsb.tile([128, 32], F32, name=f"w2b{k}") for k in range(NCH)]
            b1_sb = [wsb.tile([128, 1], F32, name=f"b1{k}") for k in range(KC)]
            msk_sb = wsb.tile([P, 1], F32, name="msk")
            nc.sync.dma_start(out=msk_sb[:], in_=hswmask[:])
            for b in range(B):
                for k in range(KC):
                    nc.sync.dma_start(out=ge_sb[b][k][:].bitcast(F32R),
                                      in_=geT[b, 128 * k:128 * (k + 1), :].bitcast(F32R))
                    nc.sync.dma_start(out=geS_sb[b][k][:].bitcast(F32R),
                                      in_=geS[b, 128 * k:128 * (k + 1), :].bitcast(F32R))
            for k in range(KC):
                for m in range(KC):
                    nc.sync.dma_start(
                        out=w1s_sb[k][m][:].bitcast(F32R),
                        in_=w1sT[128 * k:128 * (k + 1), 128 * m:128 * (m + 1)].bitcast(F32R))
                    nc.sync.dma_start(
                        out=w1t_sb[k][m][:].bitcast(F32R),
                        in_=w1tT[128 * k:128 * (k + 1), 128 * m:128 * (m + 1)].bitcast(F32R))
                # fp32 copy of W2 for the exact hsw matmul
                nc.sync.dma_start(out=w2_sb[k][:],
                                  in_=w2tp[128 * k:128 * (k + 1), :])
                nc.sync.dma_start(out=b1_sb[k][:],
                                  in_=b1c[128 * k:128 * (k + 1), :])
            for k in range(NCH):
                nc.sync.dma_start(out=w2b_sb[k][:].bitcast(F32R),
                                  in_=w2blk[k].bitcast(F32R))

            u_sb = [epool.tile([P, T], F32, name=f"u{c}") for c in range(C)]
            for c in range(C):
                nc.sync.dma_start(out=u_sb[c][:], in_=u_pl[c])

            # ---- stage A: projections ----
            htT_sb = [[hts.tile([128, T], F32, name=f"ht{b}{m}") for m in range(KC)]
                      for b in range(B)]
            hsT_sb = [[hts.tile([128, S], F32, name=f"hs{b}{m}") for m in range(KC)]
                      for b in range(B)]
            htD = dscratch.tile([B, D, T], F32, name="htD")
            hsD = dscratch.tile([B, 2, D, S], F32, name="hsD")  # [neg, pos]
            for b in range(B):
                for m in range(KC):
                    pht = psp.tile([128, T], F32, name="pht", tag="pz0")
                    for k in range(KC):
                        nc.tensor.matmul(pht[:], w1t_sb[k][m][:].bitcast(F32R),
                                         ge_sb[b][k][:].bitcast(F32R),
                                         start=(k == 0), stop=(k == KC - 1))
                    nc.scalar.activation(htT_sb[b][m][:], pht[:], AF.Identity,
                                         bias=b1_sb[m][:], scale=1.0)
                    nc.sync.dma_start(out=htD[b, 128 * m:128 * (m + 1), :],
                                      in_=htT_sb[b][m][:])
                    phs = psp.tile([128, S], F32, name="phs", tag="pz1")
                    for k in range(KC):
                        nc.tensor.matmul(phs[:], w1s_sb[k][m][:].bitcast(F32R),
                                         geS_sb[b][k][:].bitcast(F32R),
                                         start=(k == 0), stop=(k == KC - 1))
                    nc.vector.tensor_copy(hsT_sb[b][m][:], phs[:])
                    nhs = hts.tile([128, S], F32, name=f"nhs{b}{m}")
                    nc.vector.tensor_scalar(nhs[:], hsT_sb[b][m][:], -1.0, None,
                                            OP.mult)
                    nc.sync.dma_start(out=hsD[b, 0, 128 * m:128 * (m + 1), :],
                                      in_=nhs[:])
                    nc.sync.dma_start(out=hsD[b, 1, 128 * m:128 * (m + 1), :],
                                      in_=hsT_sb[b][m][:])

            # hsw[c, s] = sum_d W2[c,d] hs[s,d], exact fp32 (tiny matmul)
            hswD = dscratch.tile([C, P], F32, name="hswD")
            for b in range(B):
                phw = psp.tile([32, S], F32, name="phw", tag="pz2")
                for k in range(KC):
                    nc.tensor.matmul(phw[:], w2_sb[k][:], hsT_sb[b][k][:],
                                     start=(k == 0), stop=(k == KC - 1))
                hww = hts.tile([C, S], F32, name=f"hww{b}")
                nc.vector.tensor_copy(hww[:], phw[0:C, :])
                nc.sync.dma_start(out=hswD[:, b * S:(b + 1) * S], in_=hww[:])

            # ---- stages R+B+C+D ----
            z_t = dscratch.tile([C, P, T], F32, name="zt")
            for b in range(B):
                pz_tiles = [psp.tile([32, T], F32, name=f"pz{b}{sg}", tag=f"pz{sg}")
                            for sg in range(NG // B)]
                cols = []
                for sg in range(NG // B):
                    grp = b * (NG // B) + sg
                    neg = GROUP_ENG[grp] != "a"
                    hcol = hcolp.tile([128, NCH], F32, name=f"hcol{b}{sg}",
                                      tag="hcol")
                    # dst[p=(i,l), c16] = (+-)hs[b, sg*R+i, c16*L + l]
                    # per-instance gather: dst rows [i*L,(i+1)*L) x NCH,
                    # src hs[b, s, d] with d = c16*L + l  (layout [s, d])
                    for i in range(R):
                        s_i = sg * R + i
                        scol = hsD[b, 0 if neg else 1, :, s_i]
                        nc.sync.dma_start(
                            out=hcol[i * L:(i + 1) * L, :],
                            in_=scol.rearrange("(c16 l) -> l c16", l=L))
                    cols.append(hcol)
                for c16 in range(NCH):
                    hr = hrep.tile([128, T], F32, name="hr", tag="hr")
                    src = htD[b, c16 * L:(c16 + 1) * L, :]
                    for rep in range(R):
                        nc.sync.dma_start(out=hr[rep * L:(rep + 1) * L, :],
                                          in_=src)
                    for sg in range(NG // B):
                        grp = b * (NG // B) + sg
                        eng = GROUP_ENG[grp]
                        h = hpool.tile([128, T], F32, name="h", tag="h")
                        colbc = cols[sg][:, c16:c16 + 1].to_broadcast((128, T))
                        if eng == "a":
                            nc.scalar.activation(h[:].bitcast(F32R), hr[:],
                                                 AF.Relu,
                                                 bias=cols[sg][:, c16:c16 + 1],
                                                 scale=1.0)
                        elif eng == "v":
                            nc.vector.tensor_tensor(h[:].bitcast(F32R), hr[:],
                                                    colbc, OP.max)
                        else:
                            nc.gpsimd.tensor_tensor(h[:].bitcast(F32R), hr[:],
                                                    colbc, OP.max)
                        nc.tensor.matmul(pz_tiles[sg][0:MW, :],
                                         w2b_sb[c16][:, 0:MW].bitcast(F32R),
                                         h[:].bitcast(F32R),
                                         start=(c16 == 0), stop=(c16 == NCH - 1))
                for sg in range(NG // B):
                    grp = b * (NG // B) + sg
                    zsb = zdrain.tile([MW, T], F32, name="zsb", tag="zsb")
                    if sg % 2 == 0:
                        nc.vector.tensor_copy(zsb[:], pz_tiles[sg][0:MW, :])
                    else:
                        nc.scalar.copy(zsb[:], pz_tiles[sg][0:MW, :])
                    if sim_safe:
                        for i in range(R):
                            for c in range(C):
                                bs_i = grp * R + i
                                nc.sync.dma_start(
                                    out=z_t[c, bs_i:bs_i + 1, :],
                                    in_=zsb[C * i + c:C * i + c + 1, :])
                    else:
                        for c in range(C):
                            nc.sync.dma_start(
                                out=z_t[c, grp * R:(grp + 1) * R, :],
                                in_=zsb[c:c + C * (R - 1) + 1:C, :])

            # ---- stage E ----
            hswc = []
            for c in range(C):
                raw = epool.tile([P, 1], F32, name=f"hswraw{c}")
                nc.sync.dma_start(out=raw[:],
                                  in_=hswD[c].rearrange("(p one) -> p one", one=1))
                msked = epool.tile([P, 1], F32, name=f"hswm{c}")
                nc.vector.tensor_tensor(msked[:], raw[:], msk_sb[:], OP.mult)
                hswc.append(msked)
            q_sb = []
            for c in range(C):
                tln = epool.tile([P, T], F32, name=f"tln{c}")
                nc.scalar.activation(tln[:], u_sb[c][:], AF.Ln)
                q = epool.tile([P, T], F32, name=f"q{c}")
                nc.scalar.activation(q[:], tln[:], AF.Ln, scale=-1.0)
                q_sb.append(q)
            zt_sb = []
            for c in range(C):
                zc = epool.tile([P, T], F32, name=f"z{c}")
                nc.sync.dma_start(out=zc[:], in_=z_t[c])
                zhw = epool.tile([P, T], F32, name=f"zh{c}")
                nc.vector.tensor_tensor(zhw[:], zc[:],
                                        hswc[c][:].to_broadcast((P, T)), OP.add)
                ztc = epool.tile([P, T], F32, name=f"zt{c}")
                nc.vector.scalar_tensor_tensor(ztc[:], zhw[:], float(b2_vals[c]),
                                               q_sb[c][:], OP.add, OP.subtract)
                zt_sb.append(ztc)
            zm = epool.tile([P, T], F32, name="zm")
            nc.vector.tensor_tensor(zm[:], zt_sb[0][:], zt_sb[1][:], OP.max)
            nc.vector.tensor_tensor(zm[:], zm[:], zt_sb[2][:], OP.max)
            y_sb = []
            for c in range(C):
                yc = epool.tile([P, T], F32, name=f"y{c}")
                nc.vector.tensor_tensor(yc[:], zt_sb[c][:], zm[:], OP.is_equal)
                nc.sync.dma_start(out=y_pl[c], in_=yc[:])
                y_sb.append(yc)
            mm = epool.tile([P, T], F32, name="mmt")
            nc.vector.tensor_tensor(mm[:], y_sb[2][:], y_sb[0][:], OP.subtract)
            nc.sync.dma_start(out=m_pl[:], in_=mm[:])

    nc.finalize()
    return nc


def _build_w2blk(W2):
    # w2blk[c16][(i,l), 3i+c] = W2[c, c16*L + l]
    blk = np.zeros((NCH, 128, 32), np.float32)
    for i in range(R):
        for c in range(C):
            for c16 in range(NCH):
                blk[c16, i * L:(i + 1) * L, C * i + c] = W2[c, c16 * L:(c16 + 1) * L]
    return blk


def _prep_inputs(gene_emb, u, W1, b1, W2):
    geT = np.ascontiguousarray(gene_emb.transpose(0, 2, 1))
    w1sT = np.ascontiguousarray(W1[:, :D].T)
    w1tT = np.ascontiguousarray(W1[:, D:].T)
    w2tp = np.zeros((D, 32), np.float32)
    w2tp[:, :C] = W2.T
    w2blk = _build_w2blk(W2)
    b1col = np.ascontiguousarray(b1.reshape(D, 1))
    mask = np.zeros((P, 1), np.float32)
    for grp in range(NG):
        if GROUP_ENG[grp] != "a":
            mask[grp * R:(grp + 1) * R] = 1.0
    uP = np.ascontiguousarray(u.transpose(3, 0, 1, 2))
    in_maps = []
    for k in range(N_CORES):
        sl = slice(k * S, (k + 1) * S)
        in_maps.append({
            "geT": geT,
            "geS": np.ascontiguousarray(geT[:, :, sl]),
            "w1sT": w1sT,
            "w1tT": w1tT,
            "w2tp": w2tp,
            "w2blk": w2blk,
            "b1c": b1col,
            "hswmask": mask,
            "u_pl": np.ascontiguousarray(uP[:, :, sl, :]).reshape(C, P, T),
        })
    return in_maps


def _gather(results):
    M = np.empty((B, G, G), np.float32)
    y = np.empty((B, G, G, C), np.float32)
    for k in range(N_CORES):
        ypl = results[k]["y_pl"]
        mpl = results[k]["m_pl"]
        for b in range(B):
            M[b, k * S:(k + 1) * S] = mpl[b * S:(b + 1) * S]
            y[b, k * S:(k + 1) * S] = ypl[:, b * S:(b + 1) * S, :].transpose(1, 2, 0)
    return M, y


_prog_cache = {}


def kernel(gene_emb, u, W1, b1, W2, b2, _trace=False, _trace_kwargs=None):
    key = tuple(np.asarray(b2, np.float32).tolist())
    if key not in _prog_cache:
        _prog_cache[key] = build_program(np.asarray(b2, np.float32))
    nc = _prog_cache[key]
    in_maps = _prep_inputs(np.asarray(gene_emb, np.float32),
                           np.asarray(u, np.float32),
                           np.asarray(W1, np.float32),
                           np.asarray(b1, np.float32),
                           np.asarray(W2, np.float32))
    kw = {}
    if _trace:
        kw = dict(trace=True, **(_trace_kwargs or {}))
    res = run_bass_kernel_spmd(nc, in_maps, list(range(N_CORES)), **kw)
    M, y = _gather(res.results)
    if _trace:
        return (M, y), res
    return M, y
